# revision 1
# baseline (speedup 1.0000x reference)
"""Trainium2 Bass kernel: multi-head attention (B=4, N=2048, D=768, 12 heads).

Sharding: 8 cores = 4 batches x 2 head-groups (6 heads each).
Each core computes, for its (batch, head-group):
    qT/kT = (W[:,cols].T @ x.T)     [64*2, N] stacked head pairs
    v     = x @ Wv[:,cols]          [N, 6*64] (+ ones column per head)
    sT    = k q^T (scaled, exp'd) -> attn^T tiles [keys, queries]
    o     = e.T @ [v|1]             [queries, 65] psum chains (stationary=e)
    o_n   = o / sums  (DVE recip + broadcast multiply)
    oT    = DMA-xbar transpose of o_n -> [dims, queries]
    yT    = Wp[rows,:].T @ oT       partial output [768, N] in bf16
Host sums the two partial yT per batch (all-reduce of the row-split Wp
projection) and adds bp.

PE cost per core ~444k cycles: the e.T@v attn@v form streams N=65 per
matmul instead of N=512, halving attention-value PE time; the resulting
[query, dim] layout is transposed back via the DMA xbar (idle engines),
except the terminal block which uses PE transposes through an identity
input to keep the tail off the congested HWDGE queue.  exp work is
split between ScalarE (exact exp) and DVE (Schraudolph bf16 bit-trick);
Pool cannot access PSUM on real hardware.  attn@v lags its scores by
one block and interleaves into the next block's score groups so the
in-order PE always has fill work while exp catches up.
"""

import numpy as np
import ml_dtypes

B, N, DIM = 4, 2048, 768
HEADS, HD = 12, 64
SCALE = HD ** -0.5
NCORES = 8
HLOC = HEADS // 2        # heads per core
PAIRS = HLOC // 2        # head pairs per core
P = 128
QB = 512                 # query block
NQB = N // QB            # 4
KT = N // P              # 16 key tiles
NG = KT                  # score/exp groups per head (one key tile each:
                         # single-bank psum tiles -> reuse distance of 6
                         # buffers, so exp latency never stalls PE)
KC = DIM // P            # 6 contraction chunks for projections
VPAD = 66                # padded per-head v row (64 + ones col + 1 pad)

# Schraudolph bf16 exp on DVE for a subset of score tiles, balancing exp
# work between ScalarE (exact, 'S') and DVE (Schraudolph, 'D').  Pool
# cannot touch PSUM on real hardware, so only these two engines apply.
# Both heads of a key tile share one 2-bank psum tile, so each entry is
# a single [128, 1024] exp instruction (halves the access overhead).
EXP_ASSIGN = (
    "S", "D", "S", "D", "S", "D", "S", "D",
    "S", "D", "S", "S", "D", "S", "D", "S",
)
SCHRAU_A = 128.0 / float(np.log(2.0))      # bf16 exponent scale
SCHRAU_B = 16256.0 - 7.4                   # 127<<7 minus centering constant

_cache = {}
EPOOL_BUFS = 46          # per head tag; e tiles live ~2 blocks (lag-1 attn@v)


def _build():
    import concourse.bacc as bacc
    import concourse.mybir as mybir
    import concourse.tile as tile
    from concourse._compat import get_trn_type

    fp32 = mybir.dt.float32
    bf16 = mybir.dt.bfloat16
    fp8 = mybir.dt.float8e4
    DRow = mybir.MatmulPerfMode.DoubleRow
    i16 = mybir.dt.int16
    Exp = mybir.ActivationFunctionType.Exp
    mult = mybir.AluOpType.mult
    add = mybir.AluOpType.add

    nc = bacc.Bacc(
        get_trn_type() or "TRN2",
        target_bir_lowering=False,
        debug=False,
        enable_asserts=False,
        num_devices=NCORES,
    )

    xT = nc.dram_tensor("xT", [DIM, N], bf16, kind="ExternalInput").ap()
    wq = nc.dram_tensor("wq", [DIM, HLOC * HD], bf16, kind="ExternalInput").ap()
    wk = nc.dram_tensor("wk", [DIM, HLOC * HD], bf16, kind="ExternalInput").ap()
    wv = nc.dram_tensor("wv", [DIM, HLOC * HD], bf16, kind="ExternalInput").ap()
    wp = nc.dram_tensor("wp", [HLOC * HD, DIM], bf16, kind="ExternalInput").ap()
    ident = nc.dram_tensor("ident", [P, P], bf16, kind="ExternalInput").ap()
    yT = nc.dram_tensor("yT", [DIM, N], bf16, kind="ExternalOutput").ap()

    with tile.TileContext(nc) as tc:
        with (
            tc.tile_pool(name="const", bufs=1) as cpool,
            tc.tile_pool(name="exp", bufs=EPOOL_BUFS) as epool,
            tc.tile_pool(name="norm", bufs=4) as npool,
            tc.tile_pool(name="qkp", bufs=3, space="PSUM") as qkp,
            tc.tile_pool(name="avp", bufs=2, space="PSUM") as avp,
        ):
            # ---------------- input loads ----------------
            # whole-tensor weight loads (column-sliced loads pay the narrow-
            # row DMA penalty), ordered just-in-time against the proj order:
            # wq, x tokens 0:1024, wk, x tokens 1024:2048, wv, wp
            # kc-halved chunks let the in-order PE start each projection
            # chain just-in-time via subtile dependencies
            wq_sb = cpool.tile([P, KC, HLOC * HD], bf16, name="wq_sb")
            wqr = wq.rearrange("(o p) m -> p o m", p=P)
            xT_sb = cpool.tile([P, KC, N], bf16, name="xT_sb")
            xTr = xT.rearrange("(o p) n -> p o n", p=P)
            wk_sb = cpool.tile([P, KC, HLOC * HD], bf16, name="wk_sb")
            wkr = wk.rearrange("(o p) m -> p o m", p=P)
            nc.sync.dma_start(wq_sb[:, 0:3], wqr[:, 0:3])
            nc.sync.dma_start(xT_sb[:, 0:3, 0:QB], xTr[:, 0:3, 0:QB])
            nc.sync.dma_start(xT_sb[:, 3:6, 0:QB], xTr[:, 3:6, 0:QB])
            nc.sync.dma_start(wq_sb[:, 3:6], wqr[:, 3:6])
            nc.sync.dma_start(xT_sb[:, 0:3, QB:2 * QB], xTr[:, 0:3, QB:2 * QB])
            nc.sync.dma_start(wk_sb[:, 0:3], wkr[:, 0:3])
            nc.sync.dma_start(xT_sb[:, 3:6, QB:2 * QB], xTr[:, 3:6, QB:2 * QB])
            nc.sync.dma_start(wk_sb[:, 3:6], wkr[:, 3:6])
            nc.sync.dma_start(xT_sb[:, :, 2 * QB:3 * QB],
                              xTr[:, :, 2 * QB:3 * QB])
            nc.sync.dma_start(xT_sb[:, :, 3 * QB:], xTr[:, :, 3 * QB:])
            # wv before wp: the interleaved v-projection starts before the
            # output projection needs its weights
            wv_sb = cpool.tile([P, KC, HLOC * HD], bf16, name="wv_sb")
            nc.sync.dma_start(wv_sb, wv.rearrange("(o p) m -> p o m", p=P))
            wp_sb = cpool.tile([P, PAIRS, DIM], bf16, name="wp_sb")
            nc.sync.dma_start(wp_sb, wp.rearrange("(o p) m -> p o m", p=P))
            # identity for the terminal block's PE transposes (tail only)
            id_sb = cpool.tile([P, P], bf16, name="id_sb")
            nc.sync.dma_start(id_sb, ident)

            # HAM warm-up: dummy matmuls fill the startup DMA wait so the
            # PE clock-gate is already released (and the p-state ramp done)
            # when the projections start
            warm = cpool.tile([P, HD], bf16, name="warm")
            nc.vector.memset(warm, 0.0)
            wps = qkp.tile([P, QB], fp32, name="wps", tag="qk")
            for _w in range(105):
                nc.tensor.matmul(wps[0:HD, 0:HD], lhsT=warm[:, 0:HD], rhs=warm)

            qT_sb = [cpool.tile([P, N], bf16, name=f"qT{pr}") for pr in range(PAIRS)]
            kT_sb = [cpool.tile([P, N], bf16, name=f"kT{pr}") for pr in range(PAIRS)]
            # fp8 copies of pair0/headA q,k in DoubleRow fold layout
            # [32, 2, N] (row 2p+i); its score matmuls run at 0.5 cyc/row
            q8_sb = cpool.tile([32, 2, N], fp8, name="q8_sb")
            k8_sb = cpool.tile([32, 2, N], fp8, name="k8_sb")
            # v with a trailing ones column per head: [P, kt, head, 64+1]
            v_sb = cpool.tile([P, KT, HLOC, VPAD], bf16, name="v_sb")
            oT_sb = cpool.tile([P, PAIRS, N], bf16, name="oT_sb")
            nc.vector.memset(v_sb[:, :, :, HD:HD + 1], 1.0)

            # alternate the two fast aux engines for PSUM evacuation copies
            # (never Pool: its queue runs deep behind Schraudolph exp work and
            # a late evacuation stalls PE on the PSUM buffer reuse)
            evac_i = {"i": 0}

            def evac_copy(out, in_):
                evac_i["i"] += 1
                if evac_i["i"] % 2 == 1:
                    nc.scalar.copy(out, in_)
                else:
                    nc.vector.tensor_copy(out=out, in_=in_)

            # ---------------- emission helpers ----------------
            def emit_projqk_group(pair, wi, half, nbs=(0, 1)):
                w_sb, dst = ((wq_sb, qT_sb[pair]), (wk_sb, kT_sb[pair]))[wi]
                for nb in nbs:
                    col = half * 2 * QB + nb * QB
                    ps = qkp.tile([P, QB], fp32, name="ps_qk", tag="qk")
                    for kc in range(KC):
                        nc.tensor.matmul(
                            ps,
                            lhsT=w_sb[:, kc, pair * P:(pair + 1) * P],
                            rhs=xT_sb[:, kc, col:col + QB],
                            start=(kc == 0),
                            stop=(kc == KC - 1),
                        )
                    evac_copy(dst[:, col:col + QB], ps)

            def emit_projqk(pair, order=((0, 0), (0, 1), (1, 0), (1, 1))):
                for wi, half in order:
                    emit_projqk_group(pair, wi, half)

            def emit_projv_group(g):
                for j in range(2):
                    nt = g * 2 + j
                    ps = qkp.tile([P, QB], fp32, name="ps_v", tag="qk")
                    for kc in range(KC):
                        nc.tensor.matmul(
                            ps[:, 0:HLOC * HD],
                            lhsT=xT_sb[:, kc, nt * P:(nt + 1) * P],
                            rhs=wv_sb[:, kc, :],
                            start=(kc == 0),
                            stop=(kc == KC - 1),
                        )
                    evac_copy(
                        v_sb[:, nt, :, 0:HD],
                        ps[:, 0:HLOC * HD].rearrange("p (h d) -> p h d", d=HD),
                    )

            def emit_exp(e, ps, eng):
                if eng == "S":
                    nc.scalar.activation(e, ps, Exp, scale=SCALE)
                else:
                    nc.vector.tensor_scalar(
                        e.bitcast(i16), ps, SCALE * SCHRAU_A, SCHRAU_B,
                        mult, add)

            def emit_qk_exp(qb, pair, interleave=None, kts=None):
                """Scores + exp for one (qb, pair) block; returns e tiles."""
                qs = slice(qb * QB, (qb + 1) * QB)
                etiles = []
                for kt in (range(NG) if kts is None else kts):
                    ks = slice(kt * P, (kt + 1) * P)
                    ps2 = qkp.tile([P, 2, QB], fp32, name="ps2", tag="qk")
                    # sT[keys, queries] for the two heads of the pair,
                    # in the two 64-row halves of PE; pair0/headA runs in
                    # fp8 DoubleRow (half the streaming cycles)
                    if pair == 0 and qb != 0:
                        nc.tensor.matmul(
                            ps2[:, 0, :],
                            lhsT=k8_sb[:, :, ks],
                            rhs=q8_sb[:, :, qs],
                            perf_mode=DRow,
                            tile_position=(0, 0),
                        )
                    else:
                        nc.tensor.matmul(
                            ps2[:, 0, :],
                            lhsT=kT_sb[pair][0:HD, ks],
                            rhs=qT_sb[pair][0:HD, qs],
                            tile_position=(0, 0),
                        )
                    nc.tensor.matmul(
                        ps2[:, 1, :],
                        lhsT=kT_sb[pair][HD:P, ks],
                        rhs=qT_sb[pair][HD:P, qs],
                        tile_position=(HD, 0),
                    )
                    # one exp instruction covers both heads of the key tile
                    e2 = epool.tile([P, 2, QB], bf16, name="e2", tag="e2")
                    emit_exp(e2, ps2, EXP_ASSIGN[kt])
                    etiles.append(e2)
                    if interleave is not None:
                        interleave(kt)
                return etiles

            class AVFeeder:
                """attn@v for one block, emitted in 16-matmul chunks so it can
                interleave into the next block's score groups (keeps the
                in-order PE fed while exp catches up).  e.T @ [v|1] chains,
                then DVE norm and DMA-xbar transpose into oT_sb."""

                def __init__(self, qb, pair, etiles):
                    self.qb, self.pair, self.etiles = qb, pair, etiles
                    self.o_n = npool.tile([P, NQB, 2, HD], bf16, name="o_n",
                                          tag="on")
                    self.pos = 0          # matmuls emitted (0..2*NQB*KT)
                    self.o_ps = None

                def _norm(self, idx):
                    rec = npool.tile([P, NQB], fp32, name="rec", tag="rec")
                    nc.vector.reciprocal_approx_fast(out=rec,
                                                     in_=self.o_ps[:, :, HD])
                    nc.vector.tensor_tensor(
                        self.o_n[:, :, idx, :], self.o_ps[:, :, 0:HD],
                        rec.unsqueeze(2).broadcast_to((P, NQB, HD)), mult)

                def step(self, n=16):
                    end = min(self.pos + n, 2 * NQB * KT)
                    while self.pos < end:
                        idx, r = divmod(self.pos, NQB * KT)
                        qc, kt = divmod(r, KT)
                        if r == 0:
                            self.o_ps = avp.tile([P, NQB, HD + 1], fp32,
                                                 name="o_ps", tag="av")
                        e = self.etiles[kt]
                        nc.tensor.matmul(
                            self.o_ps[:, qc, :],
                            lhsT=e[:, idx, qc * P:(qc + 1) * P],
                            rhs=v_sb[:, kt, 2 * self.pair + idx, 0:HD + 1],
                            start=(kt == 0),
                            stop=(kt == KT - 1),
                        )
                        self.pos += 1
                        if self.pos % (NQB * KT) == 0:
                            self._norm(self.pos // (NQB * KT) - 1)
                    if self.pos == 2 * NQB * KT:
                        for qc in range(NQB):
                            nc.sync.dma_start_transpose(
                                oT_sb[:, self.pair,
                                      self.qb * QB + qc * P:
                                      self.qb * QB + (qc + 1) * P],
                                self.o_n[:, qc, :, :])
                        self.pos += 1   # mark transposed

                def finish(self):
                    if self.pos <= 2 * NQB * KT:
                        self.step(2 * NQB * KT - self.pos)

            def emit_av_last(qb, pair, etiles):
                """Terminal block, engineered for a short tail: PE transposes
                via the identity input (no HWDGE latency chain), a 2-column
                stagger, and a column-granular outproj whose stores spread
                over two DGE queues as each m finishes."""
                o_n = npool.tile([P, NQB, 2, HD], bf16, name="o_n", tag="on")
                o_A = avp.tile([P, NQB, HD + 1], fp32, name="o_A", tag="av")
                o_B = avp.tile([P, NQB, HD + 1], fp32, name="o_B", tag="av")
                # two 3-m-wide output staging tiles -> two batched stores,
                # the first of which fires while the last opcol still runs
                ysb2 = [npool.tile([P, 3, NQB, P], bf16, name=f"ysbt{g}",
                                   tag="ysbt", bufs=2) for g in range(2)]
                evac = {"i": 0}
                qs = slice(qb * QB, (qb + 1) * QB)
                yTr = yT.rearrange("(o p) n -> p o n", p=P)

                def transpose_col(qc):
                    if qc >= 2:
                        tr = avp.tile([P, P], bf16, name="tr2", tag="av")
                    else:
                        tr = qkp.tile([P, P], bf16, name="tr", tag="qk")
                    nc.tensor.matmul(tr, lhsT=o_n[:, qc, :, :], rhs=id_sb,
                                     is_transpose=True)
                    cs = slice(qb * QB + qc * P, qb * QB + (qc + 1) * P)
                    if evac["i"] % 2 == 0:
                        nc.scalar.copy(oT_sb[:, pair, cs], tr)
                    else:
                        nc.vector.tensor_copy(out=oT_sb[:, pair, cs], in_=tr)
                    evac["i"] += 1

                def opcol(qc):
                    cs = slice(qb * QB + qc * P, qb * QB + (qc + 1) * P)
                    for m in range(KC):
                        # the last two columns run after o_A/o_B are freed:
                        # alternate their psum tiles across both pools so the
                        # mini-chain rotation never waits an evacuation
                        if qc >= 2 and m % 2 == 1:
                            yps = avp.tile([P, P], fp32, name="ypc2", tag="av")
                        else:
                            yps = qkp.tile([P, P], fp32, name="ypc", tag="qk")
                        for kc in range(PAIRS):
                            nc.tensor.matmul(
                                yps,
                                lhsT=wp_sb[:, kc, m * P:(m + 1) * P],
                                rhs=oT_sb[:, kc, cs],
                                start=(kc == 0),
                                stop=(kc == PAIRS - 1),
                            )
                        g, mg = divmod(m, 3)
                        if evac["i"] % 2 == 0:
                            nc.scalar.copy(ysb2[g][:, mg, qc, :], yps)
                        else:
                            nc.vector.tensor_copy(out=ysb2[g][:, mg, qc, :],
                                                  in_=yps)
                        evac["i"] += 1
                        # progressive stores: columns go out as soon as their
                        # last evacuation lands
                        if mg == 2 and qc >= 1:
                            if qc == 1:
                                cols, span = slice(0, 2), slice(qb * QB,
                                                                qb * QB + 2 * P)
                            else:
                                cols = slice(qc, qc + 1)
                                span = slice(qb * QB + qc * P,
                                             qb * QB + (qc + 1) * P)
                            nc.sync.dma_start(
                                yTr[:, g * 3:(g + 1) * 3, span],
                                ysb2[g][:, :, cols, :].rearrange(
                                    "p m c q -> p m (c q)"))

                for qc in range(NQB):
                    for idx, o_ps in ((0, o_A), (1, o_B)):
                        for kt in range(KT):
                            e = etiles[kt]
                            nc.tensor.matmul(
                                o_ps[:, qc, :],
                                lhsT=e[:, idx, qc * P:(qc + 1) * P],
                                rhs=v_sb[:, kt, 2 * pair + idx, 0:HD + 1],
                                start=(kt == 0),
                                stop=(kt == KT - 1),
                            )
                        rec1 = npool.tile([P, 1], fp32, name="rec1", tag="rec")
                        nc.vector.reciprocal_approx_fast(
                            out=rec1, in_=o_ps[:, qc, HD:HD + 1])
                        nc.vector.tensor_tensor(
                            o_n[:, qc, idx, :], o_ps[:, qc, 0:HD],
                            rec1.broadcast_to((P, HD)), mult)
                    if qc > 0:
                        transpose_col(qc - 1)
                    if qc > 1:
                        opcol(qc - 2)
                transpose_col(NQB - 1)
                opcol(NQB - 2)
                opcol(NQB - 1)

            def emit_outproj_m(qb, m):
                qs = slice(qb * QB, (qb + 1) * QB)
                yps = avp.tile([P, QB], fp32, name="yps", tag="av")
                for kc in range(PAIRS):
                    nc.tensor.matmul(
                        yps,
                        lhsT=wp_sb[:, kc, m * P:(m + 1) * P],
                        rhs=oT_sb[:, kc, qs],
                        start=(kc == 0),
                        stop=(kc == PAIRS - 1),
                    )
                ysb = npool.tile([P, QB], bf16, name="ysb", tag="ysb")
                evac_copy(ysb, yps)
                nc.sync.dma_start(yT[m * P:(m + 1) * P, qs], ysb)

            # ---------------- schedule ----------------
            # Block order honours q/k readiness (projqk(1)/(2) interleave into
            # early blocks); attn@v lags its block's scores by one block so
            # exp never stalls PE; outproj slices drain a ready queue.
            # S(0,0)'s first half runs on token-half-0 q/k while the late xT
            # chunks are still in flight.
            vg = {"i": 0}

            def _interleave_projv(gi):
                if vg["i"] < KT // 2:
                    emit_projv_group(vg["i"])
                    vg["i"] += 1

            P2_ORDER = ((0, 0), (1, 0), (1, 1), (0, 1))
            pqk = {1: 0, 2: 0}

            def _interleave_projqk(pair):
                def hook(gi):
                    if gi % 4 == 0 and pqk[pair] < len(P2_ORDER):
                        emit_projqk_group(pair, *P2_ORDER[pqk[pair]])
                        pqk[pair] += 1
                return hook

            BLKS = [(0, 0), (1, 0), (0, 1), (0, 2), (1, 1), (1, 2),
                    (2, 0), (2, 1), (2, 2), (3, 0), (3, 1), (3, 2)]
            ILEAVE = {(1, 0): _interleave_projqk(1),
                      (0, 1): _interleave_projqk(2)}

            av_done = {qb: 0 for qb in range(NQB)}   # pairs finished per qb
            outproj_q = []                           # ready (qb, m) slices

            def note_av_done(blk):
                qb, _ = blk
                av_done[qb] += 1
                if av_done[qb] == PAIRS:
                    outproj_q.extend((qb, m) for m in range(KC))

            # preamble: q/k token-half-0 projections, then S(0,0)'s first
            # half fills the wait for the late xT chunks; the v projection
            # interleaves into the second half once wv has landed
            emit_projqk(0, order=((0, 0), (1, 0)))
            et00 = emit_qk_exp(0, 0, kts=range(NG // 2))
            emit_projqk(0, order=((1, 1), (0, 1)))
            # fp8 fold copies (SWDGE cast); DMA reads take tile-level deps,
            # so these wait for the full q/k evacuation -- hence DoubleRow
            # only serves qb>0 blocks of pair 0
            nc.gpsimd.dma_start(q8_sb, qT_sb[0][0:HD, :])
            nc.gpsimd.dma_start(k8_sb, kT_sb[0][0:HD, :])
            et00 += emit_qk_exp(0, 0, kts=range(NG // 2, NG),
                                interleave=_interleave_projv)
            while vg["i"] < KT // 2:
                emit_projv_group(vg["i"])
                vg["i"] += 1
            feeder = AVFeeder(0, 0, et00)

            # per-hook attn@v chunk sizes (16 hooks per block, 128 matmuls)
            AV_STEPS = (8, 8, 8, 8, 8, 8, 8, 8, 8, 8, 8, 8, 8, 8, 8, 8)
            AV_STEPS_LAST = (10,) * 12 + (8,) * 1 + (0,) * 3
            for bi, blk in enumerate(BLKS[1:], start=1):
                extra = ILEAVE.get(blk)
                fd = feeder
                steps = AV_STEPS_LAST if bi == len(BLKS) - 1 else AV_STEPS

                def hook(gi, extra=extra, fd=fd, steps=steps):
                    if fd is not None:
                        fd.step(steps[gi])
                    if extra is not None:
                        extra(gi)
                    # outproj slices drain late in the block so their oT
                    # transposes (issued a slot earlier) have cleared HWDGE
                    if gi >= NG - 3 and outproj_q:
                        emit_outproj_m(*outproj_q.pop(0))

                et = emit_qk_exp(*blk, interleave=hook)
                # flush any remaining interleaved proj groups
                if blk in ((1, 0), (0, 1)):
                    pair = 1 if blk == (1, 0) else 2
                    while pqk[pair] < len(P2_ORDER):
                        emit_projqk_group(pair, *P2_ORDER[pqk[pair]])
                        pqk[pair] += 1
                feeder.finish()
                note_av_done(BLKS[bi - 1])
                if bi < len(BLKS) - 1:
                    feeder = AVFeeder(*blk, et)
                else:
                    # any stragglers run while the terminal exps finish
                    while outproj_q:
                        emit_outproj_m(*outproj_q.pop(0))
                    emit_av_last(*blk, et)

    nc.compile()
    return nc


def _get_nc():
    nc = _cache.get("nc")
    if nc is None:
        nc = _build()
        _cache["nc"] = nc
    return nc


def make_in_maps(x, Wq, Wk, Wv, Wp):
    bf = ml_dtypes.bfloat16
    x = np.asarray(x, np.float32)
    Wq = np.asarray(Wq, np.float32)
    Wk = np.asarray(Wk, np.float32)
    Wv = np.asarray(Wv, np.float32)
    Wp = np.asarray(Wp, np.float32)
    xTs = [np.ascontiguousarray(x[b].T).astype(bf) for b in range(B)]
    ident = np.eye(P, dtype=bf)
    in_maps = []
    for c in range(NCORES):
        b, hg = divmod(c, 2)
        cs = slice(hg * HLOC * HD, (hg + 1) * HLOC * HD)
        in_maps.append(
            {
                "xT": xTs[b],
                "ident": ident,
                "wq": np.ascontiguousarray(Wq[:, cs]).astype(bf),
                "wk": np.ascontiguousarray(Wk[:, cs]).astype(bf),
                "wv": np.ascontiguousarray(Wv[:, cs]).astype(bf),
                "wp": np.ascontiguousarray(Wp[cs, :]).astype(bf),
            }
        )
    return in_maps


def assemble(outs, bp):
    bp32 = np.asarray(bp, np.float32)
    y = np.empty((B, N, DIM), np.float32)
    for b in range(B):
        y[b] = (outs[2 * b]["yT"].astype(np.float32)
                + outs[2 * b + 1]["yT"].astype(np.float32)).T + bp32
    return y


def kernel(x, Wq, Wk, Wv, Wp, bp):
    from concourse.bass_utils import run_bass_kernel_spmd

    nc = _get_nc()
    in_maps = make_in_maps(x, Wq, Wk, Wv, Wp)
    res = run_bass_kernel_spmd(nc, in_maps, core_ids=list(range(NCORES)))
    _cache["last_result"] = res
    return assemble(res.results, bp)

